# revision 6
# baseline (speedup 1.0000x reference)
"""Trainium2 Bass kernel for nn_Attention (general-score attention energies +
softmax over the batch axis).

Math (reference):
    proj     = einsum('lbh,oh->lbo', enc, W) + b      # [L, B, H]
    energies = einsum('bh,lbh->bl', hidden, proj)     # [B, L]
    attn     = softmax(energies, axis=0)[:, None, :]  # [B, 1, L]

Algebraic rewrite:
    energies[b, l] = (hidden @ W)[b] . enc[l, b] + hidden[b] . b
This removes the O(L*B*H*H) projection matmul; the kernel is a memory-bound
stream over enc with a tiny [B,H]x[H,H] matmul up front.

fp16 streaming: hidden/W/enc are downcast to fp16 host-side during the
shard/relayout pass, halving the dominant HBM stream (33.5 MB -> 16.8 MB
per core). All accumulation is fp32 (PE PSUM accumulate); the softmax runs
in fp32. Output rel err vs the fp32 reference ~1.6e-3 (2e-2 gate).

Distribution: enc sharded along L across 8 cores (128 l-values per core);
softmax over batch is per-l => core-local. W is sharded 8-ways along the
OUTPUT h-chunks of u = hidden @ W: core i loads only W[:, 128i:128(i+1)]
(0.26 MB instead of 2.1 MB), computes its uT chunk on PE, and a 16 KiB/core
AllGather (DRAM bounce buffers, gpsimd-triggered) replicates the full uT --
the exchange hides entirely under the enc stream thanks to a deep (12-buf)
stream pool.

Per-core dataflow (h-contraction on the TensorEngine):
  - enc shard is relaid out host-side to [128, 16*4096] fp16 with
    partition p = h%128 and free = (tile, h-chunk, l, b): every DMA tile is
    a plain 2D slice, 8 KiB contiguous per partition.
  - uT chunk: uT[128kh+p, b] = sum_o W[o, h] hidden[b, o] via 8 fp16
    matmuls (fp32 PSUM), downcast to fp16, AllGather -> full uT [h, b].
  - per tile (8 l-values): 8 accumulating matmuls
    uT_k^T [128h, 64b] @ enc_k [128h, (8l, 64b')] -> PSUM [64b, (8l, 64b')].
    The energies are the b-diagonal of each [64, 64] block: one small DVE
    scalar_tensor_tensor per l with op0=add(scalar=c2) and op1=mult
    against an fp32 identity extracts
    E[b, l] = sum_b' (P[b, l, b'] + c[b]) * I[b, b'] = P[b, l, b] + c[b]
    -- diagonal + bias in one 192 ns op with fused accum_out.
    (A direct DVE dot-product costs 1127 ns/l -- fused-accum DVE ops have
    no 2x mode -- which made DVE the bottleneck; PE contracts 5x cheaper.)
  - softmax over b in two l-halves (first half starts mid-stream):
    PE transpose [64, 64] -> reduce_max(negate) -> ScalarE exp(+bias) with
    fused row-sum -> reciprocal -> scale -> PE transpose back -> one
    DMA out [64, 128] fp32.

All pre-enc inputs (setup/W-shard/idn, 0.46 MB) go first on the same
sync-ring queue as enc so nothing gates on the slow scalar ring.

Engine budget per core: DMA 17.3 MB (~41-48 us at 360-430 GB/s), PE ~28 us,
DVE ~26 us, all overlapped => DMA-bound; ~8.6 us NEFF startup + ~2 us tail.
"""

import numpy as np

import concourse.bass as bass
import concourse.bacc as bacc
import concourse.tile as tile
from concourse import mybir
from concourse.bass_utils import run_bass_kernel_spmd

F32 = mybir.dt.float32
F16 = mybir.dt.float16

B = 64          # batch
H = 1024        # hidden dim
L = 1024        # enc_len
NCORES = 8
LS = L // NCORES            # 128 l-values per core
TILE_L = 8                  # l-values per stream tile (2 MiB fp16)
NT = LS // TILE_L           # 16 stream tiles per core
KH = H // 128               # 8 h-chunks (PE contraction dim)
MULT = mybir.AluOpType.mult
ADD = mybir.AluOpType.add


def build_program() -> bacc.Bacc:
    nc = bacc.Bacc(
        "TRN2", target_bir_lowering=False, debug=False, num_devices=NCORES
    )

    # st (fp16): cols 0:512 hidden^T chunks (st[p, 64ko+b] = hidden[b, 128ko+p])
    #            cols 512:520 bvec^T (st[p, 512+ko] = bvec[128ko+p])
    st_p = nc.declare_dram_parameter("st", [128, 520], F16, isOutput=False)
    # W shard for this core's h-chunk kh=cid:
    #   ws[p, 128ko + j] = W[o=128ko+p, h=128*cid+j]
    w_p = nc.declare_dram_parameter("w", [128, 1024], F16, isOutput=False)
    idn_p = nc.declare_dram_parameter("idn", [128, 128], F32, isOutput=False)
    # enc (fp16): enc[p, 4096t + 512k + 64c + b] = enc_shard[l=8t+c, b, h=128k+p]
    enc_p = nc.declare_dram_parameter("enc", [128, NT * 4096], F16, isOutput=False)
    out_p = nc.declare_dram_parameter("out", [B, LS], F32, isOutput=True)

    # NOTE: built as bacc.Bacc + nc.compile() -- the staged walrus rejects
    # multi-wait instructions emitted by raw Bass+Tile; bacc legalizes them.
    with tile.TileContext(nc) as tc:
        with (
            tc.tile_pool(name="const", bufs=1) as cp,
            tc.tile_pool(name="stream", bufs=12) as sp,
            tc.tile_pool(name="dram", bufs=1, space="DRAM") as dp,
            tc.tile_pool(name="pse", bufs=3, space="PSUM") as ppe,
            tc.tile_pool(name="ps1", bufs=1, space="PSUM") as pp1,
            tc.tile_pool(name="psu", bufs=1, space="PSUM") as ppu,
        ):
            # ---- pre-enc inputs, all first on the sync ring ----
            st = cp.tile([128, 520], F16)
            nc.sync.dma_start(st[:], st_p.ap())
            hT = st[:, 0:512]           # chunk ko at [:, 64ko : 64ko+64]
            bvT = st[:, 512:520]
            ws = cp.tile([128, 1024], F16)
            nc.sync.dma_start(ws[:], w_p.ap())
            idn = cp.tile([128, 128], F32)
            nc.sync.dma_start(idn[:], idn_p.ap())

            # ---- this core's uT chunk on PE: [128, 64] fp32 -> fp16 ----
            psum_ut = ppu.tile([128, B], F32, tag="psum_ut")
            for ko in range(8):
                nc.tensor.matmul(
                    psum_ut[:],
                    lhsT=ws[:, 128 * ko : 128 * (ko + 1)],
                    rhs=hT[:, 64 * ko : 64 * (ko + 1)],
                    start=(ko == 0),
                    stop=(ko == 7),
                )
            ut_chunk = cp.tile([128, B], F16)
            nc.scalar.copy(ut_chunk[:], psum_ut[:])

            # ---- AllGather uT chunks (DRAM bounce, gpsimd-triggered) ----
            ag_in = dp.tile([128, B], F16)
            ag_out = dp.tile([NCORES * 128, B], F16)
            nc.gpsimd.dma_start(ag_in[:], ut_chunk[:])
            nc.gpsimd.collective_compute(
                "AllGather",
                mybir.AluOpType.bypass,
                replica_groups=[list(range(NCORES))],
                ins=[ag_in.opt()],
                outs=[ag_out.opt()],
            )
            # uT_sb[p, 64k + b] = u[b, 128k + p] = ag_out[128k + p, b]
            uT = cp.tile([128, 8 * B], F16)
            nc.sync.dma_start(
                uT[:],
                ag_out[:].rearrange("(k p) b -> p k b", p=128),
            )

            # ---- c[b] = hidden[b] . bvec ----
            psum_c = ppu.tile([B, 1], F32, tag="psum_c")
            for ko in range(8):
                nc.tensor.matmul(
                    psum_c[:],
                    lhsT=hT[:, 64 * ko : 64 * (ko + 1)],
                    rhs=bvT[:, ko : ko + 1],
                    start=(ko == 0),
                    stop=(ko == 7),
                )
            c2 = cp.tile([B, 1], F32)
            nc.scalar.copy(c2[:], psum_c[:])

            # ---- main stream: PE contraction + DVE diag extract (+bias) ----
            ecols = cp.tile([B, LS], F32)
            scr = cp.tile([B, B], F32)   # diag STT main-out scratch
            out_sb = cp.tile([B, LS], F32)

            def softmax_half(lh):
                # softmax over b for l-columns [64*lh, 64*lh+64)
                psum_t = pp1.tile([B, B], F32, tag="pt")
                nc.tensor.transpose(
                    psum_t[:], ecols[:, B * lh : B * (lh + 1)], idn[0:B, 0:B]
                )
                negm = cp.tile([B, 1], F32, tag=f"negm{lh}")
                nc.vector.tensor_reduce(
                    out=negm[:],
                    in_=psum_t[:],
                    axis=mybir.AxisListType.X,
                    op=mybir.AluOpType.max,
                    negate=True,
                )
                pexp = cp.tile([B, B], F32, tag=f"pexp{lh}")
                ssum = cp.tile([B, 1], F32, tag=f"ssum{lh}")
                nc.scalar.activation(
                    pexp[:],
                    psum_t[:],
                    mybir.ActivationFunctionType.Exp,
                    bias=negm[:, 0:1],
                    scale=1.0,
                    accum_out=ssum[:],
                )
                rs = cp.tile([B, 1], F32, tag=f"rs{lh}")
                nc.vector.reciprocal(rs[:], ssum[:])
                attn = cp.tile([B, B], F32, tag=f"attn{lh}")
                nc.vector.tensor_scalar_mul(attn[:], pexp[:], rs[:, 0:1])
                psum_o = pp1.tile([B, B], F32, tag="po")
                nc.tensor.transpose(psum_o[:], attn[:], idn[0:B, 0:B])
                nc.vector.tensor_copy(out_sb[:, B * lh : B * (lh + 1)], psum_o[:])

            for t in range(NT):
                et = sp.tile([128, 4096], F16, tag="et")
                nc.sync.dma_start(
                    et[:], enc_p.ap()[:, 4096 * t : 4096 * (t + 1)]
                )
                pe_t = ppe.tile([B, 512], F32, tag="pe")
                for k in range(KH):
                    nc.tensor.matmul(
                        pe_t[:],
                        lhsT=uT[:, 64 * k : 64 * (k + 1)],
                        rhs=et[:, 512 * k : 512 * (k + 1)],
                        start=(k == 0),
                        stop=(k == KH - 1),
                    )
                for c in range(TILE_L):
                    l = TILE_L * t + c
                    # accum = sum_b' (P[b, l, b'] + c2[b]) * I[b, b']
                    #       = P[b, l, b] + c2[b]   (diag + bias in one op)
                    nc.vector.scalar_tensor_tensor(
                        out=scr[:],
                        in0=pe_t[:, B * c : B * (c + 1)],
                        scalar=c2[:, 0:1],
                        in1=idn[0:B, 0:B],
                        op0=ADD,
                        op1=MULT,
                        accum_out=ecols[:, l : l + 1],
                    )
                if t == NT // 2 - 1:
                    softmax_half(0)   # l 0:64 complete; runs mid-stream
            softmax_half(1)
            nc.sync.dma_start(out_p.ap(), out_sb[:])

    nc.compile()
    return nc


_IDENT = np.eye(128, dtype=np.float32)
_NC_CACHE = []


def _get_nc() -> bacc.Bacc:
    if not _NC_CACHE:
        _NC_CACHE.append(build_program())
    return _NC_CACHE[0]


def make_in_maps(hidden, encoder_outputs, W, b):
    # layout transforms + fp16 downcast done host-side during sharding;
    # all FLOPs (matmuls, energy contraction, softmax) stay on device
    hidden16 = np.asarray(hidden, dtype=np.float16)
    # st[p, 64ko + b] = hidden[b, 128ko + p]
    hTp = hidden16.T.reshape(8, 128, B).transpose(1, 0, 2).reshape(128, 512)
    bvT = np.asarray(b, dtype=np.float16).reshape(8, 128).T  # [128, 8]
    st = np.ascontiguousarray(np.concatenate([hTp, bvT], axis=1))
    # per-core W shard: ws_i[p, 128ko + j] = W[128ko + p, 128i + j]
    W16 = np.asarray(W, dtype=np.float16)
    wt = W16.reshape(8, 128, 8, 128).transpose(1, 2, 0, 3)  # [p, kh, ko, j]
    in_maps = []
    for i in range(NCORES):
        ws = np.ascontiguousarray(wt[:, i].reshape(128, 1024))
        shard16 = encoder_outputs[i * LS : (i + 1) * LS].astype(np.float16)
        # [l, b, h] -> [p = h%128, (t, k, c, b)] with l = 8t+c, h = 128k+p
        enc_pc = np.ascontiguousarray(
            shard16.reshape(NT, TILE_L, B, KH, 128)
            .transpose(4, 0, 3, 1, 2)
            .reshape(128, NT * 4096)
        )
        in_maps.append({"st": st, "idn": _IDENT, "enc": enc_pc, "w": ws})
    return in_maps


def kernel(hidden, encoder_outputs, W, b):
    nc = _get_nc()
    in_maps = make_in_maps(hidden, encoder_outputs, W, b)
    res = run_bass_kernel_spmd(nc, in_maps, core_ids=list(range(NCORES)))
    out = np.concatenate([res.results[i]["out"] for i in range(NCORES)], axis=1)
    return out[:, None, :].astype(np.float32)
